# revision 6
# baseline (speedup 1.0000x reference)
"""DenseCoAttn Trainium2 kernel (8 NeuronCores, batch-parallel).

Problem: B=32, L=512, DIM=1024, H=8, DK=128, NN=3 none-tokens.
  v_s = concat(none_s, value_s); q_s = v_s @ W_s.T  (s in {1,2})
  w1 = attn(q=q2, k=q1, v=q1, mask=m1)[:, NN:, :]
  w2 = attn(q=q1, k=q2, v=q2, mask=m2)[:, NN:, :]

Sharding: data-parallel over batch, 4 batches per core, no collectives.

Per-core kernel layout choices:
  * token order: kt 0..511 = value tokens, 512..514 = none tokens,
    515..639 = zero padding.  (attention is permutation-invariant over
    keys; queries are value tokens 0..511 only, since the reference
    slices [NN:] off the query axis.)
  * projections are computed feature-major q_fm[d, t] (d on partitions,
    one 128-wide chunk per head) with f32r matmuls: lhsT = W^T chunk,
    rhs = v^T chunk.  v^T / W^T / none^T are prepared host-side
    (layout prep only, zero FLOPs on host).
  * scores are computed transposed, S^T[kt, qt] = K_fm_chunk^T @ Q_fm,
    so softmax's sum over kt becomes a matmul contraction.
  * masking is folded into V: V rows of masked keys (and the fused
    denominator ones-column) are multiplied by the 0/1 mask during
    V construction, so exp() needs no bias and masked keys contribute
    exactly 0 to both numerator and denominator (matches the
    reference's -1e9 bias whose exp underflows to exactly 0).
  * V token-major tiles are built by PE-transposing q_fm 128x128
    blocks; stored fp16 with a per-head ones/mask column appended at
    col 128 -> PV matmul (pexp stationary, [V|mask] streaming, fp16)
    accumulates both O_unnorm and the softmax denominator.
  * normalize: per-partition reciprocal + multiply on DVE.
"""

import os
import numpy as np

import concourse.bass as bass
import concourse.mybir as mybir
import concourse.tile as tile
from concourse import bacc
from concourse.bass_utils import run_bass_kernel_spmd

F32 = mybir.dt.float32
F32R = mybir.dt.float32r
F16 = mybir.dt.float16
I32 = mybir.dt.int32
EXP = mybir.ActivationFunctionType.Exp

P = 128
NCORES = 8
BPC = 4            # batches per core
L = 512            # value tokens
D = 1024
H = 8              # heads == dout chunks
KC = 8             # k (contraction) chunks
NN = 3             # none tokens
TQ = 515           # 512 values + 3 none (no padding)
QT = 4             # query chunks of 128
KT = 5             # key chunks of 128 (incl. none+pad chunk)
SCALE = float(1.0 / np.sqrt(128.0))


def build_module(reps: int = 1):
    nc = bacc.Bacc("TRN2", target_bir_lowering=False)

    # ---- DRAM IO (per-core shard shapes) ----
    vt1 = nc.dram_tensor("vt1", [BPC, KC, P, L], F32R, kind="ExternalInput")
    vt2 = nc.dram_tensor("vt2", [BPC, KC, P, L], F32R, kind="ExternalInput")
    w1t = nc.dram_tensor("w1t", [KC, P, D], F32R, kind="ExternalInput")
    w2t = nc.dram_tensor("w2t", [KC, P, D], F32R, kind="ExternalInput")
    n1t = nc.dram_tensor("n1t", [KC, P, 4], F32R, kind="ExternalInput")
    n2t = nc.dram_tensor("n2t", [KC, P, 4], F32R, kind="ExternalInput")
    m1s = nc.dram_tensor("m1s", [BPC, P, QT], I32, kind="ExternalInput")
    m2s = nc.dram_tensor("m2s", [BPC, P, QT], I32, kind="ExternalInput")
    ident = nc.dram_tensor("ident", [P, P], F32R, kind="ExternalInput")
    w1o = nc.dram_tensor("w1o", [BPC, L, D], F32, kind="ExternalOutput")
    w2o = nc.dram_tensor("w2o", [BPC, L, D], F32, kind="ExternalOutput")

    vts = (vt1, vt2)
    wts = (w1t, w2t)
    nts = (n1t, n2t)
    mss = (m1s, m2s)
    wos = (w1o, w2o)

    with tile.TileContext(nc) as tc:
        with tc.tile_pool(name="const", bufs=1) as const_pool, \
             tc.tile_pool(name="io", bufs=1) as io_pool, \
             tc.tile_pool(name="work", bufs=1) as work_pool, \
             tc.tile_pool(name="psum", bufs=1, space="PSUM") as psum_pool:

            pools = (const_pool, io_pool, work_pool, psum_pool)
            tensors = (vts, wts, nts, mss, wos, ident)
            if reps == 1:
                _emit(nc, pools, tensors)
            else:
                # timing builds: run the whole per-invocation body `reps`
                # times inside one NEFF so device time dominates dispatch
                with tc.For_i(0, reps, 1,
                              hint_engines=(mybir.EngineType.PE,
                                            mybir.EngineType.DVE,
                                            mybir.EngineType.Activation,
                                            mybir.EngineType.SP)):
                    _emit(nc, pools, tensors)

    nc.compile()
    return nc


def _emit(nc, pools, tensors):
    const_pool, io_pool, work_pool, psum_pool = pools
    vts, wts, nts, mss, wos, ident = tensors
    if True:
        if True:
            # ---- constants ----
            w_sb = []
            nt_sb = []
            for s in range(2):
                wsb = const_pool.tile([P, KC, D], F32R, tag=f"w{s}", bufs=1,
                                      name=f"w{s}_sb")
                nc.sync.dma_start(wsb[:], wts[s][:].rearrange("k p d -> p k d"))
                w_sb.append(wsb)
                nsb = const_pool.tile([P, KC, 4], F32R, tag=f"n{s}", bufs=1,
                                      name=f"n{s}_sb")
                nc.sync.dma_start(nsb[:], nts[s][:].rearrange("k p d -> p k d"))
                nt_sb.append(nsb)
            id_sb = const_pool.tile([P, P], F32R, tag="ident", bufs=1,
                                    name="id_sb")
            nc.sync.dma_start(id_sb[:], ident[:])

            # none-token feature-major projections (built during batch 0)
            nfm_sb = [
                const_pool.tile([P, H, NN], F32R, tag=f"nfm{s}", bufs=1,
                                name=f"nfm{s}_sb")
                for s in range(2)
            ]
            # none-token V rows (kt chunk 4): [3 tokens x (heads x 129)]
            v4_sb = [
                const_pool.tile([NN, H, P + 1], F16, tag=f"v4_{s}", bufs=1,
                                name=f"v4_{s}_sb")
                for s in range(2)
            ]

            for b in range(BPC):
                qfm = [[None] * H, [None] * H]   # [side][head] -> AP
                vtm = [[None] * QT, [None] * QT]  # [side][tc] -> AP
                msk = [None, None]

                # ================= projections (+V build) per side ======
                for s in range(2):
                    vt_sb = io_pool.tile([P, KC, L], F32R, tag="vt", bufs=2,
                                         name=f"vt_b{b}s{s}")
                    nc.sync.dma_start(
                        vt_sb[:], vts[s][b].rearrange("k p t -> p k t"))

                    # mask -> float 0/1, kt-partition layout
                    mi = io_pool.tile([P, QT], I32, tag="mski", bufs=2,
                                      name=f"mi_b{b}s{s}")
                    nc.sync.dma_start(mi[:], mss[s][b])
                    mf = io_pool.tile([P, QT], F32, tag="mskf", bufs=4,
                                      name=f"mf_b{b}s{s}")
                    nc.vector.tensor_copy(mf[:], mi[:])
                    msk[s] = mf

                    for dc in range(H):
                        pp = psum_pool.tile([P, L], F32, tag="mm", bufs=2,
                                            name=f"pp_b{b}s{s}d{dc}")
                        if b == 0:
                            pn = psum_pool.tile([P, 4], F32, tag="s", bufs=2,
                                                name=f"pn_s{s}d{dc}")
                        for kc in range(KC):
                            lhsT = w_sb[s][:, kc, dc * P:(dc + 1) * P]
                            nc.tensor.matmul(pp[:], lhsT, vt_sb[:, kc, :],
                                             start=(kc == 0), stop=(kc == KC - 1))
                            if b == 0:
                                nc.tensor.matmul(pn[:], lhsT, nt_sb[s][:, kc, :],
                                                 start=(kc == 0),
                                                 stop=(kc == KC - 1))
                        if b == 0:
                            nc.vector.tensor_copy(nfm_sb[s][:, dc, :], pn[:, 0:NN])

                        qf = work_pool.tile([P, TQ], F32R, tag="qfm", bufs=16,
                                            name=f"qf_b{b}s{s}d{dc}")
                        nc.vector.tensor_copy(qf[:, 0:L], pp[:])
                        nc.vector.tensor_copy(qf[:, L:L + NN], nfm_sb[s][:, dc, :])
                        qfm[s][dc] = qf

                    # ---- V build: PE-transpose q_fm blocks, mask folded ----
                    for tch in range(QT):
                        vt_t = work_pool.tile([P, H, P + 1], F16, tag="vtm",
                                              bufs=8, name=f"vtm_b{b}s{s}t{tch}")
                        for dc in range(H):
                            pt = psum_pool.tile([P, P], F32R, tag="mm", bufs=2,
                                                name=f"pt_b{b}s{s}t{tch}d{dc}")
                            nc.tensor.transpose(
                                pt[:], qfm[s][dc][:, tch * P:(tch + 1) * P],
                                id_sb[:])
                            # V[kt, d] * mask[kt]
                            nc.vector.tensor_scalar(
                                vt_t[:, dc, 0:P], pt[:],
                                mf[:, tch:tch + 1], None,
                                mybir.AluOpType.mult)
                        # ones/mask column for the fused denominator
                        nc.vector.tensor_copy(
                            vt_t[:, :, P:P + 1],
                            mf[:, tch:tch + 1, None].to_broadcast((P, H, 1)))
                        vtm[s][tch] = vt_t

                    # ---- none V rows (once per core) ----
                    if b == 0:
                        for dc in range(H):
                            pt4 = psum_pool.tile([P, P], F32R, tag="mm", bufs=2,
                                                 name=f"pt4_s{s}d{dc}")
                            nc.tensor.transpose(pt4[0:NN, :],
                                                nfm_sb[s][:, dc, :], id_sb[:])
                            nc.vector.tensor_copy(v4_sb[s][:, dc, 0:P],
                                                  pt4[0:NN, :])
                        # ones column via x*0+1 (avoids memset ISA issues)
                        nc.vector.tensor_scalar(
                            v4_sb[s][:, :, P:P + 1], v4_sb[s][:, :, 0:1],
                            0.0, 1.0, mybir.AluOpType.mult,
                            mybir.AluOpType.add)

                # ================= attentions ===========================
                # attn index a: a=0 -> out w1: Q=side1(q2)? NO:
                #   w1 = attn(q=q2, k=q1, v=q1): K/V side 0, Q side 1.
                #   w2 = attn(q=q1, k=q2, v=q2): K/V side 1, Q side 0.
                for a in range(2):
                    kv, qs = (0, 1) if a == 0 else (1, 0)
                    outst = [
                        io_pool.tile([P, D], F32, tag="outs", bufs=5,
                                     name=f"o_b{b}a{a}q{qtc}")
                        for qtc in range(QT)
                    ]
                    for h in range(H):
                        kf = qfm[kv][h]
                        qf = qfm[qs][h]
                        # scores^T + exp, in ktc pairs
                        pexps = []
                        for pair in range(3):
                            sps = psum_pool.tile([P, 1024], F32, tag="s",
                                                 bufs=2,
                                                 name=f"s_b{b}a{a}h{h}p{pair}")
                            if pair == 2:
                                nc.tensor.matmul(
                                    sps[0:NN, 0:512],
                                    kf[:, L:L + NN],
                                    qf[:, 0:L],
                                    start=True, stop=True)
                                pe = work_pool.tile([P, 1024], F16, tag="pexp",
                                                    bufs=4,
                                                    name=f"pe_b{b}a{a}h{h}p{pair}")
                                nc.scalar.activation(pe[0:NN, 0:512],
                                                     sps[0:NN, 0:512],
                                                     EXP, scale=SCALE)
                            else:
                                for i in range(2):
                                    ktc = pair * 2 + i
                                    nc.tensor.matmul(
                                        sps[:, i * 512:(i + 1) * 512],
                                        kf[:, ktc * P:(ktc + 1) * P],
                                        qf[:, 0:L],
                                        start=True, stop=True)
                                pe = work_pool.tile([P, 1024], F16, tag="pexp",
                                                    bufs=4,
                                                    name=f"pe_b{b}a{a}h{h}p{pair}")
                                nc.scalar.activation(pe[:, 0:1024],
                                                     sps[:, 0:1024],
                                                     EXP, scale=SCALE)
                            pexps.append(pe)
                        # PV with fused denominator (qtc-outer: one open
                        # accumulation group per PSUM bank at a time)
                        for qtc in range(QT):
                            op = psum_pool.tile([P, P + 1], F32, tag="o",
                                                bufs=2,
                                                name=f"op_b{b}a{a}h{h}q{qtc}")
                            for ktc in range(KT):
                                pe = pexps[ktc // 2]
                                off = (ktc % 2) * 512
                                if ktc == KT - 1:
                                    lhsT = pe[0:NN, qtc * P:(qtc + 1) * P]
                                    rhs = v4_sb[kv][:, h, :]
                                else:
                                    lhsT = pe[:, off + qtc * P:
                                              off + (qtc + 1) * P]
                                    rhs = vtm[kv][ktc][:, h, :]
                                nc.tensor.matmul(
                                    op[:], lhsT, rhs,
                                    start=(ktc == 0), stop=(ktc == KT - 1))
                            rc = work_pool.tile([P, 1], F32, tag="rcp", bufs=4,
                                                name=f"rc_b{b}a{a}h{h}q{qtc}")
                            nc.vector.reciprocal(rc[:], op[:, P:P + 1])
                            nc.vector.tensor_scalar(
                                outst[qtc][:, h * P:(h + 1) * P],
                                op[:, 0:P], rc[:], None,
                                mybir.AluOpType.mult)
                    for qtc in range(QT):
                        nc.sync.dma_start(
                            wos[a][b, qtc * P:(qtc + 1) * P, :], outst[qtc][:])


_CACHE = {}


def _get_nc():
    if "nc" not in _CACHE:
        _CACHE["nc"] = build_module()
    return _CACHE["nc"]


def _prep_in_maps(value1, value2, mask1, mask2, W1, W2, none_emb1, none_emb2):
    """Host-side layout prep (slicing / transposition only, no FLOPs)."""
    value1 = np.asarray(value1, dtype=np.float32)
    value2 = np.asarray(value2, dtype=np.float32)
    mask1 = np.asarray(mask1, dtype=np.int32)
    mask2 = np.asarray(mask2, dtype=np.int32)
    W1 = np.asarray(W1, dtype=np.float32)
    W2 = np.asarray(W2, dtype=np.float32)
    none_emb1 = np.asarray(none_emb1, dtype=np.float32)
    none_emb2 = np.asarray(none_emb2, dtype=np.float32)

    B = value1.shape[0]
    assert B == NCORES * BPC

    # [B, L, D] -> [B, KC, P, L]  (k-major transposed values)
    def vprep(v):
        return np.ascontiguousarray(
            v.reshape(B, L, KC, P).transpose(0, 2, 3, 1))

    # [D, D] -> [KC, P, D]  (W^T with k chunked onto partitions)
    def wprep(w):
        return np.ascontiguousarray(w.T.reshape(KC, P, D))

    # [NN, D] -> [KC, P, 4]  (zero-padded 4th col: f32r matmul needs N%4==0)
    def nprep(n):
        nt = np.zeros((D, 4), dtype=np.float32)
        nt[:, :NN] = n.T
        return np.ascontiguousarray(nt.reshape(KC, P, 4))

    # [B, L] -> [B, P, QT]  (kt-partition swizzle: kt = c*128 + p)
    def mprep(m):
        return np.ascontiguousarray(m.reshape(B, QT, P).transpose(0, 2, 1))

    vt1 = vprep(value1)
    vt2 = vprep(value2)
    m1 = mprep(mask1)
    m2 = mprep(mask2)
    w1t = wprep(W1)
    w2t = wprep(W2)
    n1t = nprep(none_emb1)
    n2t = nprep(none_emb2)
    eye = np.eye(P, dtype=np.float32)

    in_maps = []
    for c in range(NCORES):
        sl = slice(c * BPC, (c + 1) * BPC)
        in_maps.append({
            "vt1": vt1[sl], "vt2": vt2[sl],
            "m1s": m1[sl], "m2s": m2[sl],
            "w1t": w1t, "w2t": w2t,
            "n1t": n1t, "n2t": n2t,
            "ident": eye,
        })
    return in_maps


def kernel(value1, value2, mask1, mask2, W1, W2, none_emb1, none_emb2):
    nc = _get_nc()
    in_maps = _prep_in_maps(value1, value2, mask1, mask2,
                            W1, W2, none_emb1, none_emb2)
    res = run_bass_kernel_spmd(nc, in_maps, core_ids=list(range(NCORES)))
    _CACHE["last_results"] = res
    w1 = np.concatenate([res.results[c]["w1o"] for c in range(NCORES)], axis=0)
    w2 = np.concatenate([res.results[c]["w2o"] for c in range(NCORES)], axis=0)
    return (w1, w2)
